# revision 8
# baseline (speedup 1.0000x reference)
"""Trainium2 Bass kernel for nn_MidAttnBlock: GN+SiLU -> qkv 1x1 conv ->
4-head LxL attention -> GN -> proj -> GN+SiLU -> conv3 -> residual.

Sharding: data-parallel over batch, one sample per NeuronCore (B=8, 8 cores).
Everything for a sample stays resident in SBUF; attention runs flash-style
(scores tile in PSUM -> exp on ScalarE -> V@exp accumulated in PSUM) without
materializing the 2048x2048 attention matrix off-chip. Softmax normalization:
scores are tiny (|s|<<1) so exp needs no max subtraction; the denominator
comes for free from a ones-row appended to V^T, and the divide happens on the
96x2048 output instead of the 2048x2048 matrix.
"""

import numpy as np

C = 96          # channels
L = 2048        # sequence length
B = 8           # batch (one sample per core)
H = 4           # heads
DQK = 48        # per-head q/k channels
JB = 512        # fp32 matmul moving-operand / psum-bank width
JH = 1024       # attention j-half width (2 psum banks)
NB = L // JB
NI = L // 128   # key-tile count
EPS = 1e-5
G1 = 32         # gn1/gn3: 32 groups of 3 channels
G2 = 8          # gn2: 8 groups of 12 channels per head
MAGIC = 0x5F3759DF

_CACHE = {}


def _patch_tile_tail():
    """This toolchain's walrus accepts at most one sync-wait on a Drain
    (CTRL) instruction, but TileContext's tail drain carries one wait per
    DMA-queue/engine semaphore. Split them across a chain of 1-wait drains."""
    import bass_rust
    import concourse.tile as tile
    from concourse.vector_clock import ScopedClock

    if getattr(tile.TileContext, "_drain_patched", False):
        return

    def _split_multiwaits(nc):
        """Walrus here allows only one sync-wait slot on most instruction
        encodings. Move excess semaphore waits to preceding same-engine
        NoOps (same stream position => identical blocking semantics)."""
        from concourse import mybir

        for f in nc.m.functions:
            for bb in f.blocks:
                insts = bb.instructions
                out = []
                changed = False
                for inst in insts:
                    si = inst.sync_info
                    if si is not None:
                        waits = list(si.on_wait)
                        sem_waits = [w for w in waits if w.sync_type == "semaphore"]
                        if len(waits) > 1 and len(sem_waits) >= 1:
                            keep = [w for w in waits if w.sync_type != "semaphore"]
                            movable = sem_waits[:-1] if not keep else sem_waits
                            kept_sem = sem_waits[-1:] if not keep else []
                            for idx, w in enumerate(movable):
                                nop = mybir.InstNoOp(
                                    name=f"{inst.name}_w{idx}",
                                    engine=inst.engine,
                                    bass_nofuse=True,
                                    sync_info=bass_rust.SyncInfo(
                                        on_wait=[w], on_update=[]
                                    ),
                                )
                                out.append(nop)
                            inst.sync_info = bass_rust.SyncInfo(
                                on_wait=keep + kept_sem,
                                on_update=list(si.on_update),
                            )
                            changed = True
                    out.append(inst)
                if changed:
                    bb.instructions = out

    def _drain_and_barrier(self, tick_clock, wait_clock):
        drain_inst = self.nc.sync.drain()
        wait_clock.add_sem_waits(
            drain_inst.ins, ScopedClock({None: tick_clock.global_clock})
        )
        self.nc.all_engine_barrier()
        assert self.sems is not None
        popped = self.nc._tile_sem_poison_stack.pop()
        assert popped is self._sem_poison
        self.nc.clear_and_free_semaphores(list(self.sems.allocated().values()))
        self.nc.all_engine_barrier()
        _split_multiwaits(self.nc)

    tile.TileContext._drain_and_barrier = _drain_and_barrier
    tile.TileContext._drain_patched = True


def _build(scale_f):
    from contextlib import ExitStack

    import concourse.bass as bass
    import concourse.tile as tile
    from concourse import mybir

    _patch_tile_tail()

    f32 = mybir.dt.float32
    u32 = mybir.dt.uint32
    Alu = mybir.AluOpType
    Act = mybir.ActivationFunctionType
    AX = mybir.AxisListType

    nc = bass.Bass()

    x_d = nc.dram_tensor("x", [C, L], f32, kind="ExternalInput")
    wq_d = nc.dram_tensor("wq", [C, H, DQK], f32, kind="ExternalInput")
    wk_d = nc.dram_tensor("wk", [C, H, DQK], f32, kind="ExternalInput")
    wv_d = nc.dram_tensor("wv", [C, H * C], f32, kind="ExternalInput")
    vb_d = nc.dram_tensor("vb", [128, H, C], f32, kind="ExternalInput")
    pw_d = nc.dram_tensor("pw", [C, H, C], f32, kind="ExternalInput")
    cw_d = nc.dram_tensor("cw", [C, 3, C], f32, kind="ExternalInput")
    i1_d = nc.dram_tensor("i1", [C, G1], f32, kind="ExternalInput")
    i1t_d = nc.dram_tensor("i1t", [G1, C], f32, kind="ExternalInput")
    i2_d = nc.dram_tensor("i2", [C, G2], f32, kind="ExternalInput")
    i2t_d = nc.dram_tensor("i2t", [G2, C], f32, kind="ExternalInput")
    aux_d = nc.dram_tensor("aux", [C, 22], f32, kind="ExternalInput")
    out_d = nc.dram_tensor("out", [C, L], f32, kind="ExternalOutput")

    with tile.TileContext(nc) as tc, ExitStack() as ctx:
        P_const = ctx.enter_context(tc.tile_pool(name="const", bufs=1))
        P_scr = ctx.enter_context(tc.tile_pool(name="scr", bufs=2))
        P_e = ctx.enter_context(tc.tile_pool(name="e", bufs=2))
        P_rb = ctx.enter_context(tc.tile_pool(name="rb", bufs=2))
        P_row = ctx.enter_context(tc.tile_pool(name="row", bufs=1))
        P_small = ctx.enter_context(tc.tile_pool(name="small", bufs=1))
        P_tiny = ctx.enter_context(tc.tile_pool(name="tiny", bufs=8))
        pp = ctx.enter_context(tc.tile_pool(name="pp", bufs=2, space="PSUM"))
        pa = ctx.enter_context(tc.tile_pool(name="pa", bufs=2, space="PSUM"))

        # ---- persistent SBUF tensors ----
        sb_x = P_const.tile([C, L], f32, tag="x", name="x")
        sb_q = P_const.tile([DQK, H, L], f32, tag="q", name="q")
        sb_k = P_const.tile([DQK, H, L], f32, tag="k", name="k")
        sb_vt = P_const.tile([128, NI, H, DQK * 2 + 1], f32, tag="vt", name="vt")
        sb_O = P_const.tile([C, H, L], f32, tag="O", name="O")
        sb_s = P_const.tile([C, L + 4], f32, tag="s", name="s")
        sb_wq = P_const.tile([C, H, DQK], f32, tag="wq", name="wq")
        sb_wk = P_const.tile([C, H, DQK], f32, tag="wk", name="wk")
        sb_wv = P_const.tile([C, H * C], f32, tag="wv", name="wv")
        sb_vb = P_const.tile([128, H, C], f32, tag="vb", name="vb")
        sb_pw = P_const.tile([C, H, C], f32, tag="pw", name="pw")
        sb_cw = P_const.tile([C, 3, C], f32, tag="cw", name="cw")
        sb_i1 = P_const.tile([C, G1], f32, tag="i1", name="i1")
        sb_i1t = P_const.tile([G1, C], f32, tag="i1t", name="i1t")
        sb_i2 = P_const.tile([C, G2], f32, tag="i2", name="i2")
        sb_i2t = P_const.tile([G2, C], f32, tag="i2t", name="i2t")
        sb_aux = P_const.tile([C, 22], f32, tag="aux", name="aux")
        sb_magic = P_const.tile([128, 1], u32, tag="magic", name="magic")
        sb_c15 = P_const.tile([128, 1], f32, tag="c15", name="c15")

        nc.sync.dma_start(out=sb_x, in_=x_d[:])
        nc.sync.dma_start(out=sb_wq, in_=wq_d[:])
        nc.sync.dma_start(out=sb_wk, in_=wk_d[:])
        nc.sync.dma_start(out=sb_wv, in_=wv_d[:])
        nc.sync.dma_start(out=sb_vb, in_=vb_d[:])
        nc.sync.dma_start(out=sb_pw, in_=pw_d[:])
        nc.sync.dma_start(out=sb_cw, in_=cw_d[:])
        nc.sync.dma_start(out=sb_i1, in_=i1_d[:])
        nc.sync.dma_start(out=sb_i1t, in_=i1t_d[:])
        nc.sync.dma_start(out=sb_i2, in_=i2_d[:])
        nc.sync.dma_start(out=sb_i2t, in_=i2t_d[:])
        nc.sync.dma_start(out=sb_aux, in_=aux_d[:])

        nc.vector.memset(sb_magic, MAGIC)
        nc.vector.memset(sb_c15, 1.5)
        nc.vector.memset(sb_vt[:, :, :, DQK * 2 : DQK * 2 + 1], 1.0)
        nc.vector.memset(sb_s[:, 0:1], 0.0)
        nc.vector.memset(sb_s[:, L + 1 : L + 4], 0.0)

        def tiny(p, n, dt, tag):
            return P_tiny.tile([p, n], dt, tag=tag, name=tag)

        def quake_rsqrt(P, vep, name):
            """rstd = 1/sqrt(vep), Newton x3 from the bit-hack seed."""
            sh = tiny(P, 1, u32, "qk_sh")
            nc.vector.tensor_scalar(
                sh, vep.bitcast(u32), 1, None, op0=Alu.logical_shift_right
            )
            y = tiny(P, 1, f32, "qk_y")
            nc.vector.tensor_tensor(y.bitcast(u32), sb_magic[0:P], sh, op=Alu.subtract)
            vh = tiny(P, 1, f32, "qk_vh")
            nc.vector.tensor_scalar(vh, vep, 0.5, None, op0=Alu.mult)
            for it in range(3):
                t = tiny(P, 1, f32, f"qk_t{it}")
                nc.vector.tensor_tensor(t, y, y, op=Alu.mult)
                nc.vector.tensor_tensor(t, t, vh, op=Alu.mult)
                nc.vector.tensor_tensor(t, sb_c15[0:P], t, op=Alu.subtract)
                yn = tiny(P, 1, f32, f"qk_yn{it}")
                nc.vector.tensor_tensor(yn, y, t, op=Alu.mult)
                y = yn
            return y

        def gn_affine(G, ind, indt, st, w_col, b_col, name):
            """From raw (sum, sumsq) rows in st [C,2], group-reduce via
            indicator matmuls; returns (a, nb) with y = x*a - nb."""
            gst_ps = pp.tile([G, 2], f32, tag="big", name="big")
            nc.tensor.matmul(gst_ps, ind, st, start=True, stop=True)
            gst = tiny(G, 2, f32, f"gst_{name}")
            nc.vector.tensor_copy(gst, gst_ps)
            msq = tiny(G, 1, f32, f"msq_{name}")
            nc.vector.tensor_mul(msq, gst[:, 0:1], gst[:, 0:1])
            vep = tiny(G, 1, f32, f"vep_{name}")
            nc.vector.tensor_tensor(vep, gst[:, 1:2], msq, op=Alu.subtract)
            nc.vector.tensor_scalar(vep, vep, EPS, None, op0=Alu.add)
            rstd = quake_rsqrt(G, vep, name)
            gml = tiny(G, 2, f32, f"gml_{name}")
            nc.vector.tensor_copy(gml[:, 0:1], gst[:, 0:1])
            nc.vector.tensor_copy(gml[:, 1:2], rstd)
            cm_ps = pp.tile([C, 2], f32, tag="big", name="big")
            nc.tensor.matmul(cm_ps, indt, gml, start=True, stop=True)
            a = P_small.tile([C, 1], f32, tag=f"a_{name}", name=f"a_{name}")
            nc.vector.tensor_mul(a, cm_ps[:, 1:2], w_col)
            nb = P_small.tile([C, 1], f32, tag=f"nb_{name}", name=f"nb_{name}")
            nc.vector.scalar_tensor_tensor(
                nb, cm_ps[:, 0:1], a, b_col, op0=Alu.mult, op1=Alu.subtract
            )
            return a, nb

        # ---- stage B: GN1 + silu (tanh form; the 1/2 is folded into wq/wk/wv)
        st1 = P_small.tile([C, 2], f32, tag="st1", name="st1")
        nc.vector.reduce_sum(out=st1[:, 0:1], in_=sb_x, axis=AX.X)
        junk1 = P_scr.tile([128, L + 4], f32, tag="scr", name="scr")
        nc.vector.scalar_tensor_tensor(
            junk1[0:C, 0:L], sb_x, 1.0, sb_x, op0=Alu.mult, op1=Alu.mult,
            accum_out=st1[:, 1:2],
        )
        a1, nb1 = gn_affine(G1, sb_i1, sb_i1t, st1, sb_aux[:, 0:1], sb_aux[:, 1:2], "gn1")
        y1 = P_scr.tile([128, L + 4], f32, tag="scr", name="scr")
        nc.vector.tensor_scalar(
            y1[0:C, 0:L], sb_x, a1, nb1, op0=Alu.mult, op1=Alu.subtract
        )
        u1 = P_scr.tile([128, L + 4], f32, tag="scr", name="scr")
        nc.scalar.activation(u1[0:C, 0:L], y1[0:C, 0:L], Act.Tanh, scale=0.5)
        nc.vector.scalar_tensor_tensor(
            u1[0:C, 0:L], u1[0:C, 0:L], 1.0, y1[0:C, 0:L], op0=Alu.add, op1=Alu.mult
        )
        hbuf = u1[0:C, 0:L]

        # ---- stage C: q/k projections (per head, M=48) ----
        for h in range(H):
            for wt, bcol, dst in (
                (sb_wq, sb_aux[0:DQK, 6 + h : 7 + h], sb_q),
                (sb_wk, sb_aux[0:DQK, 10 + h : 11 + h], sb_k),
            ):
                lhs = wt[:, h, :]
                for jj in range(2):
                    ps = pp.tile([DQK, JH], f32, tag="big", name="big")
                    nc.tensor.matmul(
                        ps[:, 0:JB], lhs, hbuf[:, jj * JH : jj * JH + JB],
                        start=True, stop=True,
                    )
                    nc.tensor.matmul(
                        ps[:, JB:JH], lhs, hbuf[:, jj * JH + JB : (jj + 1) * JH],
                        start=True, stop=True,
                    )
                    nc.vector.tensor_scalar(
                        dst[:, h, jj * JH : (jj + 1) * JH], ps, bcol, None, op0=Alu.add
                    )

        # ---- stage D: v^T tiles (h as stationary operand transposes v) ----
        for i in range(NI):
            ps = pp.tile([128, H, C], f32, tag="big", name="big")
            nc.tensor.matmul(
                ps, hbuf[:, i * 128 : (i + 1) * 128], sb_wv, start=True, stop=True
            )
            nc.vector.tensor_tensor(
                sb_vt[:, i, :, 0:C], ps, sb_vb, op=Alu.add
            )

        # ---- stage E: attention per (head, j-half) ----
        cs_dram = [
            nc.dram_tensor(f"cs{i}", [JH], f32, kind="Internal")
            for i in range(2 * H * 2)
        ]
        for h in range(H):
            for jh in range(2):
                j0 = jh * JH
                o_ps = pa.tile([128, JH], f32, tag="acc", name="acc")
                for i in range(NI):
                    sc = pp.tile([128, JH], f32, tag="big", name="big")
                    lk = sb_k[:, h, i * 128 : (i + 1) * 128]
                    nc.tensor.matmul(
                        sc[:, 0:JB], lk, sb_q[:, h, j0 : j0 + JB],
                        start=True, stop=True,
                    )
                    nc.tensor.matmul(
                        sc[:, JB:JH], lk, sb_q[:, h, j0 + JB : j0 + JH],
                        start=True, stop=True,
                    )
                    e = P_e.tile([128, JH], f32, tag="e", name="e")
                    nc.scalar.activation(e, sc, Act.Exp, scale=scale_f)
                    lv = sb_vt[:, i, h, :]
                    nc.tensor.matmul(
                        o_ps[0 : C + 1, 0:JB], lv, e[:, 0:JB],
                        start=(i == 0), stop=(i == NI - 1),
                    )
                    nc.tensor.matmul(
                        o_ps[0 : C + 1, JB:JH], lv, e[:, JB:JH],
                        start=(i == 0), stop=(i == NI - 1),
                    )
                # softmax denominator: row C of o_ps is sum_l exp(s[l, m])
                rowt = P_row.tile([1, JH], f32, tag="row", name="row")
                nc.vector.tensor_copy(rowt, o_ps[C : C + 1, :])
                d1 = cs_dram[2 * (2 * h + jh)]
                nc.sync.dma_start(out=d1[:], in_=rowt)
                rt = P_row.tile([128, 8], f32, tag="rt", name="rt")
                nc.sync.dma_start(out=rt, in_=d1.rearrange("(p f) -> p f", p=128))
                rr = P_row.tile([128, 8], f32, tag="rr", name="rr")
                nc.vector.reciprocal(rr, rt)
                d2 = cs_dram[2 * (2 * h + jh) + 1]
                nc.sync.dma_start(out=d2.rearrange("(p f) -> p f", p=128), in_=rr)
                rb = P_rb.tile([C, JH], f32, tag="rb", name="rb")
                d2ap = d2[:]
                bc_ap = type(d2ap)(tensor=d2ap.tensor, offset=d2ap.offset,
                                   ap=[[0, C]] + list(d2ap.ap))
                nc.sync.dma_start(out=rb, in_=bc_ap)
                nc.vector.tensor_mul(sb_O[:, h, j0 : j0 + JH], o_ps[0:C, :], rb)

        # ---- stage F: GN2 (8 groups of 12 channels per head) ----
        for h in range(H):
            st2 = P_small.tile([C, 2], f32, tag=f"st2_{h}", name=f"st2_{h}")
            nc.vector.reduce_sum(out=st2[:, 0:1], in_=sb_O[:, h, :], axis=AX.X)
            junk2 = P_scr.tile([128, L + 4], f32, tag="scr", name="scr")
            nc.vector.scalar_tensor_tensor(
                junk2[0:C, 0:L], sb_O[:, h, :], 1.0, sb_O[:, h, :],
                op0=Alu.mult, op1=Alu.mult, accum_out=st2[:, 1:2],
            )
            a2, nb2 = gn_affine(
                G2, sb_i2, sb_i2t, st2,
                sb_aux[:, 14 + h : 15 + h], sb_aux[:, 18 + h : 19 + h], f"gn2_{h}",
            )
            nc.vector.tensor_scalar(
                sb_O[:, h, :], sb_O[:, h, :], a2, nb2, op0=Alu.mult, op1=Alu.subtract
            )

        # ---- stage G: proj (contract 384 = 4 head-chunks of 96) ----
        p_t = P_scr.tile([128, L + 4], f32, tag="scr", name="scr")
        st3p = P_small.tile([C, NB], f32, tag="st3p", name="st3p")
        for j in range(NB):
            ps = pp.tile([C, JB], f32, tag="big", name="big")
            for k in range(H):
                nc.tensor.matmul(
                    ps, sb_pw[:, k, :], sb_O[:, k, j * JB : (j + 1) * JB],
                    start=(k == 0), stop=(k == H - 1),
                )
            nc.vector.tensor_scalar(
                p_t[0:C, j * JB : (j + 1) * JB], ps, sb_aux[:, 4:5], 0.0,
                op0=Alu.add, op1=Alu.add, accum_out=st3p[:, j : j + 1],
            )

        # ---- stage H: GN3 + silu into padded buffer ----
        st3 = P_small.tile([C, 2], f32, tag="st3", name="st3")
        nc.vector.reduce_sum(out=st3[:, 0:1], in_=st3p, axis=AX.X)
        junk3 = P_scr.tile([128, L + 4], f32, tag="scr", name="scr")
        nc.vector.scalar_tensor_tensor(
            junk3[0:C, 0:L], p_t[0:C, 0:L], 1.0, p_t[0:C, 0:L],
            op0=Alu.mult, op1=Alu.mult, accum_out=st3[:, 1:2],
        )
        a3, nb3 = gn_affine(G1, sb_i1, sb_i1t, st3, sb_aux[:, 2:3], sb_aux[:, 3:4], "gn3")
        nc.vector.tensor_scalar(
            p_t[0:C, 0:L], p_t[0:C, 0:L], a3, nb3, op0=Alu.mult, op1=Alu.subtract
        )
        y3 = p_t
        u3 = P_scr.tile([128, L + 4], f32, tag="scr", name="scr")
        nc.scalar.activation(u3[0:C, 0:L], y3[0:C, 0:L], Act.Tanh, scale=0.5)
        nc.vector.scalar_tensor_tensor(
            sb_s[:, 1 : L + 1], u3[0:C, 0:L], 1.0, y3[0:C, 0:L],
            op0=Alu.add, op1=Alu.mult,
        )

        # ---- stage I: conv3 (3 shifted matmuls) + bias + residual + store ----
        for j in range(NB):
            ps = pp.tile([C, JB], f32, tag="big", name="big")
            for t in range(3):
                nc.tensor.matmul(
                    ps, sb_cw[:, t, :], sb_s[:, j * JB + t : j * JB + t + JB],
                    start=(t == 0), stop=(t == 2),
                )
            ot = P_rb.tile([C, JB], f32, tag="rb", name="rb")
            nc.vector.scalar_tensor_tensor(
                ot, ps, sb_aux[:, 5:6], sb_x[:, j * JB : (j + 1) * JB],
                op0=Alu.add, op1=Alu.add,
            )
            nc.sync.dma_start(out=out_d[:, j * JB : (j + 1) * JB], in_=ot)

    return nc


def _prep_inputs(inputs):
    x = np.asarray(inputs["x"], dtype=np.float32)
    qkv_w = np.asarray(inputs["qkv_w"], dtype=np.float32)
    qkv_b = np.asarray(inputs["qkv_b"], dtype=np.float32)
    proj_w = np.asarray(inputs["proj_w"], dtype=np.float32)
    proj_b = np.asarray(inputs["proj_b"], dtype=np.float32)
    conv_w = np.asarray(inputs["conv3_w"], dtype=np.float32)
    conv_b = np.asarray(inputs["conv3_b"], dtype=np.float32)
    gn1_w = np.asarray(inputs["gn1_w"], dtype=np.float32)
    gn1_b = np.asarray(inputs["gn1_b"], dtype=np.float32)
    gn2_w = np.asarray(inputs["gn2_w"], dtype=np.float32)
    gn2_b = np.asarray(inputs["gn2_b"], dtype=np.float32)
    gn3_w = np.asarray(inputs["gn3_w"], dtype=np.float32)
    gn3_b = np.asarray(inputs["gn3_b"], dtype=np.float32)
    scale_f = float(np.asarray(inputs["scale"]))

    wq = np.empty((C, H, DQK), np.float32)
    wk = np.empty((C, H, DQK), np.float32)
    wv = np.empty((C, H * C), np.float32)
    qb = np.zeros((C, H), np.float32)
    kb = np.zeros((C, H), np.float32)
    for h in range(H):
        wq[:, h, :] = 0.5 * qkv_w[DQK * h : DQK * (h + 1), :].T
        wk[:, h, :] = 0.5 * qkv_w[192 + DQK * h : 192 + DQK * (h + 1), :].T
        wv[:, C * h : C * (h + 1)] = 0.5 * qkv_w[384 + C * h : 384 + C * (h + 1), :].T
        qb[0:DQK, h] = qkv_b[DQK * h : DQK * (h + 1)]
        kb[0:DQK, h] = qkv_b[192 + DQK * h : 192 + DQK * (h + 1)]
    vb = np.broadcast_to(
        qkv_b[384:768].reshape(1, H, C), (128, H, C)
    ).copy().astype(np.float32)
    pw = np.empty((C, H, C), np.float32)
    for k in range(H):
        pw[:, k, :] = proj_w[:, C * k : C * (k + 1)].T
    cw = np.empty((C, 3, C), np.float32)
    for t in range(3):
        cw[:, t, :] = 0.5 * conv_w[:, :, t].T

    cidx = np.arange(C)
    i1 = (cidx[:, None] // 3 == np.arange(G1)[None, :]).astype(np.float32) / (3 * L)
    i1t = (cidx[:, None] // 3 == np.arange(G1)[None, :]).T.astype(np.float32).copy()
    i2 = (cidx[:, None] // 12 == np.arange(G2)[None, :]).astype(np.float32) / (12 * L)
    i2t = (cidx[:, None] // 12 == np.arange(G2)[None, :]).T.astype(np.float32).copy()

    aux = np.zeros((C, 22), np.float32)
    aux[:, 0] = gn1_w
    aux[:, 1] = gn1_b
    aux[:, 2] = gn3_w
    aux[:, 3] = gn3_b
    aux[:, 4] = proj_b
    aux[:, 5] = conv_b
    aux[:, 6:10] = qb
    aux[:, 10:14] = kb
    for h in range(H):
        aux[:, 14 + h] = gn2_w[C * h : C * (h + 1)]
        aux[:, 18 + h] = gn2_b[C * h : C * (h + 1)]

    shared = {
        "wq": wq, "wk": wk, "wv": wv, "vb": vb, "pw": pw, "cw": cw,
        "i1": i1, "i1t": i1t, "i2": i2, "i2t": i2t, "aux": aux,
    }
    in_maps = [dict(shared, x=np.ascontiguousarray(x[i])) for i in range(B)]
    return in_maps, scale_f


def kernel(_trace=False, _trace_kwargs=None, **inputs):
    from concourse.bass_utils import run_bass_kernel_spmd

    in_maps, scale_f = _prep_inputs(inputs)
    if scale_f not in _CACHE:
        _CACHE[scale_f] = _build(scale_f)
    nc = _CACHE[scale_f]
    res = run_bass_kernel_spmd(
        nc, in_maps, core_ids=list(range(B)), trace=_trace,
        **(_trace_kwargs or {}),
    )
    out = np.stack([res.results[i]["out"] for i in range(B)], axis=0)
    if _trace:
        return out, res
    return out
